# revision 57
# baseline (speedup 1.0000x reference)
"""Trainium2 Bass kernel for multi-head self-attention (B=2, N=2048, DIM=1024,
16 heads x 64). Sharding: core i handles batch b=i//4 and 4 heads hg=i%4
(tensor-parallel on heads: column-shard Wq/Wkv, row-shard Wo; partial outputs
summed on host).

Math notes:
  - `similarity` adds a per-query constant along the softmax axis, so softmax
    is invariant to it -> it is accepted but unused.
  - Softmax computed without max-subtraction (logits are O(10); exp is safe in
    fp32) as exp(dots)/Z with Z obtained for free as a 65th "ones" column of V
    in the E@V matmul.
  - Everything is computed transposed (q^T, k^T laid out [d, n]) so no
    on-device transposes are needed anywhere.
"""

import os
import sys

import numpy as np

sys.path.insert(0, "/opt/trn_rl_repo")

import ml_dtypes

B, N, DIM = 2, 2048, 1024
HEADS, DHEAD = 16, 64
HG = 4  # heads per core
SCALE = DHEAD**-0.5
NCORES = 8
P = 128
NI = 512  # i-chunk (matmul moving free dim)
NIC = N // NI  # 4 i-chunks
NJT = N // P  # 16 j tiles
CT = DIM // P  # 8 contraction tiles
GJ = 3  # j-tiles per dots psum staging group

LAST_RESULTS = None
_CACHED_NC = None
_EXP_OPS = None


def _get_exp_ops():
    """Register (once) two custom-DVE ops that together compute
    exp(SCALE*x) as (1 + SCALE*x/4096)^4096 — an affine seed + 12
    squarings split 4+8 across two 8-stage DVE instructions. Offloading a
    few exp tiles per chunk to the DVE relieves the scalar engine, whose
    ACT throughput (with per-call overhead) is the chunk-phase bottleneck.
    Max rel err ~0.8% at |logit|~7.5 (one-in-a-million tail), p99 0.4%."""
    global _EXP_OPS
    if _EXP_OPS is not None:
        return _EXP_OPS
    import numpy as np
    from concourse import dve_ops as DO
    from concourse.dve_spec import C0, C1, Spec, Src0, sq
    from concourse.dve_table_gen import dve_ver_for

    def _ref_seed(in0, in1, s0, s1, imm2):
        y = (np.float32(s1) + in0.astype(np.float32) * np.float32(s0)).astype(
            np.float32
        )
        for _ in range(4):
            y = (y * y).astype(np.float32)
        return y

    def _ref_sq8(in0, in1, s0, s1, imm2):
        y = in0.astype(np.float32)
        for _ in range(8):
            y = (y * y).astype(np.float32)
        return y

    seed = DO.DveOp(
        "EXP_SEED_SQ4_ANT",
        Spec(body=sq(sq(sq(sq(C1 + Src0 * C0)))), reference=_ref_seed),
        subdim=False,
        uops_sha={"v3": "d493cc7732c8af28"},
    )
    sq8 = DO.DveOp(
        "EXP_SQ8_ANT",
        Spec(body=sq(sq(sq(sq(sq(sq(sq(sq(Src0)))))))), reference=_ref_sq8),
        subdim=False,
        uops_sha={"v3": "c290ddcdd0487e61"},
    )
    ver = dve_ver_for("TRN2")
    for op in (seed, sq8):
        if op.name not in DO._SUB_OPCODE_FOR_NAME:
            DO.OPS.append(op)
            DO.CUSTOM_DVE_SPECS[op.name] = op.spec
            DO._SUB_OPCODE_FOR_NAME[op.name] = (
                max(DO._SUB_OPCODE_FOR_NAME.values()) + 1
            )
        op.compile(ver)  # sha-checked
    _EXP_OPS = (seed, sq8)
    return _EXP_OPS


def _ensure_profile_hook():
    """Provide antenv.axon_hooks (absent in this image) so that
    run_bass_kernel_spmd(trace=True) can NTFF-profile through axon."""
    import contextlib
    import ctypes
    import types

    try:
        import antenv.axon_hooks  # noqa: F401

        return
    except ImportError:
        pass
    if "antenv.axon_hooks" in sys.modules:
        return
    mod = types.ModuleType("antenv.axon_hooks")
    state = {"hook": None}
    mod.set_axon_ntff_profile_hook = lambda h: state.__setitem__("hook", h)
    mod.get_axon_ntff_profile_hook = lambda: state["hook"]
    sys.modules["antenv.axon_hooks"] = mod
    try:
        import antenv

        antenv.axon_hooks = mod
    except ImportError:
        pass

    so_path = "/opt/axon/libaxon_pjrt.so"
    if not os.path.exists(so_path):
        return
    try:
        lib = ctypes.CDLL(so_path)
    except OSError:
        return
    if not hasattr(lib, "axon_start_nrt_profile"):
        return
    lib.axon_start_nrt_profile.argtypes = [
        ctypes.POINTER(ctypes.c_int64),
        ctypes.c_size_t,
    ]
    lib.axon_start_nrt_profile.restype = ctypes.c_int64
    lib.axon_stop_nrt_profile.argtypes = [ctypes.c_char_p]
    lib.axon_stop_nrt_profile.restype = ctypes.c_int64

    @contextlib.contextmanager
    def _hook(output_dir, device_ids):
        import jax

        jax.devices()
        if device_ids:
            ids = (ctypes.c_int64 * len(device_ids))(*device_ids)
            rc = lib.axon_start_nrt_profile(ids, len(device_ids))
        else:
            rc = lib.axon_start_nrt_profile(None, 0)
        if rc != 0:
            raise RuntimeError(f"axon_start_nrt_profile rc={rc}")
        try:
            yield
        finally:
            n = lib.axon_stop_nrt_profile(str(output_dir).encode())
            print(f"ntff profile: {n} file(s) written to {output_dir}")

    mod.set_axon_ntff_profile_hook(_hook)


def _build_program():
    import concourse.tile as tile
    from concourse import bacc, mybir

    f32 = mybir.dt.float32
    bf16 = mybir.dt.bfloat16
    Exp = mybir.ActivationFunctionType.Exp

    nc = bacc.Bacc("TRN2", target_bir_lowering=False, debug=False)
    xT = nc.dram_tensor("xT", [DIM, N], bf16, kind="ExternalInput").ap()
    wq = nc.dram_tensor("wq", [DIM, HG * DHEAD], bf16, kind="ExternalInput").ap()
    wk = nc.dram_tensor("wk", [DIM, HG * DHEAD], bf16, kind="ExternalInput").ap()
    wv = nc.dram_tensor("wv", [DIM, HG * DHEAD], bf16, kind="ExternalInput").ap()
    wo = nc.dram_tensor("wo", [HG * DHEAD, DIM], bf16, kind="ExternalInput").ap()
    out = nc.dram_tensor("out", [N, DIM], bf16, kind="ExternalOutput").ap()

    with tile.TileContext(nc) as tc:
        _emit(tc, nc, mybir, out, xT, wq, wk, wv, wo, f32, bf16, Exp)
    nc.compile()
    return nc


def _emit(tc, nc, mybir, out, xT, wq, wk, wv, wo, f32, bf16, Exp):
    from collections import deque

    exp_seed, exp_sq8 = _get_exp_ops()
    # exp tiles computed on DVE instead of ACT. Measured on HW: every
    # offload variant ((6,10,14)/(9,14), (12,15)) ran 3-13us SLOWER than
    # none — the in-order PE ends up stalling on DVE-produced E tiles
    # (custom-op exp is 2.4us/tile vs ACT's 1.15) and the DVE FIFO delays
    # the norm chain behind them. Keep empty; ops stay registered for the
    # reciprocal_approx_fast path.
    DVE_EXP_JTS = {0: (), 1: ()}

    with (
        tc.tile_pool(name="cpool", bufs=1) as cpool,
        tc.tile_pool(name="apool", bufs=2, space="PSUM") as apool,
        tc.tile_pool(name="gpool", bufs=2, space="PSUM") as gpool,
        tc.tile_pool(name="dpool", bufs=2, space="PSUM") as dpool,
        tc.tile_pool(name="epool", bufs=2) as epool,
        tc.tile_pool(name="wpool", bufs=2) as wpool,
        tc.tile_pool(name="opool", bufs=8) as opool,
        tc.tile_pool(name="drpool", bufs=2, space="DRAM") as drpool,
    ):
        # ---- constants. DMA order matters: V-projection work is first, so
        # wv and the first x^T column-chunk lead both queues; wq/wk/wo are
        # needed only ~35us in.
        wv_sb = cpool.tile([P, CT, 256], bf16, name="wv_sb")
        nc.scalar.dma_start(wv_sb[:], wv.rearrange("(t p) m -> p t m", p=P))
        xt = cpool.tile([P, CT, N], bf16, name="xt")
        wq_sb = cpool.tile([P, CT, 256], bf16, name="wq_sb")
        wk_sb = cpool.tile([P, CT, 256], bf16, name="wk_sb")
        wo_sb = cpool.tile([P, 2, DIM], bf16, name="wo_sb")

        # x^T streamed as one descriptor per i-chunk on the sync queue (chunk
        # 0 transfers first, concurrent with the weights on the scalar
        # queue, ordered by consumption: V-proj first, then K, Q, O)
        # cc0 and cc1 split across the sync+gpsimd queues (V-proj consumes a
        # chunk every ~3.4us — one queue alone streams ~1MB in ~8us and
        # stalls it); cc2 on sync, cc3 rides the scalar queue behind wv/wk.
        xt_src = xT.rearrange("(t p) m -> p t m", p=P)
        for cc in range(2):
            s = slice(cc * NI, (cc + 1) * NI)
            nc.sync.dma_start(xt[:, 0:4, s], xt_src[:, 0:4, s])
            nc.gpsimd.dma_start(xt[:, 4:8, s], xt_src[:, 4:8, s])
        nc.sync.dma_start(
            xt[:, :, 2 * NI : 3 * NI], xt_src[:, :, 2 * NI : 3 * NI]
        )
        nc.scalar.dma_start(wk_sb[:], wk.rearrange("(t p) m -> p t m", p=P))
        nc.scalar.dma_start(
            xt[:, :, 3 * NI : 4 * NI], xt_src[:, :, 3 * NI : 4 * NI]
        )
        nc.scalar.dma_start(wq_sb[:], wq.rearrange("(t p) m -> p t m", p=P))
        nc.scalar.dma_start(wo_sb[:], wo.rearrange("(t p) m -> p t m", p=P))

        # Q^T, K^T [256, N] as 2 partition-tiles; V padded to 128 cols per
        # head: [v(64) | ones(1) | zeros(63)] so lhsT is 128 wide (FWL).
        QT = cpool.tile([P, 2, N], bf16, name="QT")
        KT = cpool.tile([P, 2, N], bf16, name="KT")
        Vo = cpool.tile([P, NJT, HG * 65], bf16, name="Vo")
        Vo_heads = Vo.rearrange("p j (h c) -> p j h c", c=65)
        nc.vector.memset(Vo_heads[:, :, :, 64:65], 1.0)
        ones_sb = cpool.tile([P, 64], bf16, name="ones_sb")
        nc.vector.memset(ones_sb[:], 1.0)

        # ---- emission helpers ----
        # Projection / output-projection matmuls are emitted as GENERATORS
        # that yield after each matmul. The chunk loop pumps them BETWEEN its
        # jt iterations: the PE executes in-order, and exp (scalar) is slower
        # per chunk than the chunk's own matmuls, so each E@V would stall
        # ~0.3us/jt waiting on its exp unless filler matmuls sit between.
        fill_queue = deque()  # projection generators (safe to pump any time)
        late_queue = deque()  # outproj generators (gated on a norm chain)

        def pump(k, late_ok):
            while k > 0:
                if fill_queue:
                    q = fill_queue
                elif late_ok and late_queue:
                    q = late_queue
                else:
                    return
                try:
                    next(q[0])
                    k -= 1
                except StopIteration:
                    q.popleft()

        def drain_all():
            while fill_queue or late_queue:
                pump(1, True)

        def gen_proj_q(pt, ic):
            q_ps = gpool.tile([P, NI], f32, tag="work", name="q_ps")
            for ct in range(CT):
                nc.tensor.matmul(
                    q_ps,
                    lhsT=wq_sb[:, ct, pt * P : (pt + 1) * P],
                    rhs=xt[:, ct, ic * NI : (ic + 1) * NI],
                    start=(ct == 0),
                    stop=(ct == CT - 1),
                )
                yield
            nc.vector.tensor_copy(out=QT[:, pt, ic * NI : (ic + 1) * NI], in_=q_ps)

        def gen_proj_k(pt, ic):
            k_ps = gpool.tile([P, NI], f32, tag="work", name="k_ps")
            for ct in range(CT):
                nc.tensor.matmul(
                    k_ps,
                    lhsT=wk_sb[:, ct, pt * P : (pt + 1) * P],
                    rhs=xt[:, ct, ic * NI : (ic + 1) * NI],
                    start=(ct == 0),
                    stop=(ct == CT - 1),
                )
                yield
            nc.vector.tensor_copy(out=KT[:, pt, ic * NI : (ic + 1) * NI], in_=k_ps)

        def emit_now(gen):
            for _ in gen:
                pass

        def gen_proj_v(jt):
            v_ps = gpool.tile([P, 256], f32, tag="work", name="v_ps")
            for ct in range(CT):
                nc.tensor.matmul(
                    v_ps,
                    lhsT=xt[:, ct, jt * P : (jt + 1) * P],
                    rhs=wv_sb[:, ct, :],
                    start=(ct == 0),
                    stop=(ct == CT - 1),
                )
                yield
            nc.vector.tensor_copy(
                out=Vo_heads[:, jt, :, 0:64],
                in_=v_ps.rearrange("p (h c) -> p h c", h=HG),
            )

        def emit_proj_v(jt):
            emit_now(gen_proj_v(jt))

        def emit_chunk_mms(pt, ic):
            """dots -> exp -> E@V accumulation for one (head-pair, i-chunk)."""
            hA, hB = 2 * pt, 2 * pt + 1
            # the final chunk keeps every exp on ACT so its DVE queue is
            # clear for the tail's norm chain (DVE is strict FIFO)
            last = pt == 1 and ic == NIC - 1
            dve_jts = () if last else DVE_EXP_JTS[pt]
            E = epool.tile([P, 2, NJT, NI], bf16, tag="E", name="E")
            otA = apool.tile([65, NI], f32, tag="acc", name="otA")
            otB = apool.tile([65, NI], f32, tag="acc", name="otB")

            def emit_ot(jt):
                nc.tensor.matmul(
                    otA,
                    lhsT=Vo_heads[:, jt, hA, :],
                    rhs=E[:, 0, jt, :],
                    start=(jt == 0),
                    stop=(jt == NJT - 1),
                )
                nc.tensor.matmul(
                    otB,
                    lhsT=Vo_heads[:, jt, hB, :],
                    rhs=E[:, 1, jt, :],
                    start=(jt == 0),
                    stop=(jt == NJT - 1),
                )

            # software-pipelined: E@V for jt-1 is emitted after dots for jt,
            # so the PE never stalls on the exp of the tile it just produced.
            # Fills are pumped between jts; outproj fills (late_queue) only
            # in the second half, after the previous chunk's norm chain
            # (emitted just before this chunk) has had ~8us to complete.
            for jt in range(NJT):
                dAB = dpool.tile([P, 2, NI], f32, tag="dAB", name="dAB")
                nc.tensor.matmul(
                    dAB[:, 0, :],
                    lhsT=KT[0:64, pt, jt * P : (jt + 1) * P],
                    rhs=QT[0:64, pt, ic * NI : (ic + 1) * NI],
                    start=True,
                    stop=True,
                )
                nc.tensor.matmul(
                    dAB[:, 1, :],
                    lhsT=KT[64:128, pt, jt * P : (jt + 1) * P],
                    rhs=QT[64:128, pt, ic * NI : (ic + 1) * NI],
                    start=True,
                    stop=True,
                )
                if jt in dve_jts:
                    ex = wpool.tile(
                        [P, 2 * NI], f32, tag="exf", name="ex", bufs=2
                    )
                    nc.vector._custom_dve(
                        exp_seed, out=ex[:], in0=dAB[:], s0=SCALE / 4096.0, s1=1.0
                    )
                    nc.vector._custom_dve(exp_sq8, out=E[:, :, jt, :], in0=ex[:])
                else:
                    nc.scalar.activation(
                        out=E[:, :, jt, :], in_=dAB[:], func=Exp, scale=SCALE
                    )
                if jt > 0:
                    emit_ot(jt - 1)
                if jt < NJT // 2:
                    pump(1, False)
                else:
                    # the final chunk pumps its trailing outproj at half
                    # rate: ~8 matmuls stay queued for the post-loop, where
                    # they overlap the last norm chain instead of stretching
                    # this (scalar-bound) chunk past the exp rate
                    pump(1 if last else 3, True)
            emit_ot(NJT - 1)
            return otA, otB

        onp_tiles = {}

        def emit_norm_front(pt, ic, otA, otB, last=False):
            """DVE-only part: drain Z/O out of PSUM (frees acc slots) and
            compute 1/Z. Returns state for emit_norm_back. For the last
            chunk, the cross-partition hops ride the scalar DMA queue (idle
            then) instead of sync (backed up with out-writes)."""
            hop = nc.scalar if last else nc.sync
            zrow = wpool.tile([65, 2 * NI], f32, tag="zrow", name="zrow")
            nc.vector.tensor_copy(out=zrow[64:65, 0:NI], in_=otA[64:65, :])
            if last:
                # scalar is idle at the tail: run the second Z copy there,
                # in parallel with the DVE's first
                nc.scalar.activation(
                    out=zrow[64:65, NI : 2 * NI],
                    in_=otB[64:65, :],
                    func=mybir.ActivationFunctionType.Copy,
                )
            else:
                nc.vector.tensor_copy(
                    out=zrow[64:65, NI : 2 * NI], in_=otB[64:65, :]
                )
            # hop Z to partition 0 first: the custom-DVE approx reciprocal
            # computes garbage on HW for APs at a nonzero partition offset
            # (and partition_broadcast reads partition 0 anyway)
            z0 = wpool.tile([1, 2 * NI], f32, tag="z0", name="z0", bufs=2)
            hop.dma_start(z0[:], zrow[64:65, :])
            zi0f = wpool.tile([1, 2 * NI], f32, tag="zi0f", name="zi0f", bufs=2)
            nc.vector.reciprocal_approx_fast(out=zi0f[:], in_=z0[:])
            zi0 = wpool.tile([1, 2 * NI], bf16, tag="zi0", name="zi0", bufs=2)
            nc.vector.tensor_copy(out=zi0[:], in_=zi0f[:])
            tmpB = wpool.tile([64, NI], bf16, tag="tmp", name="tmpB", bufs=4)
            nc.vector.tensor_copy(out=tmpB, in_=otB[0:64, :])
            tmpA = wpool.tile([64, NI], bf16, tag="tmp", name="tmpA", bufs=4)
            nc.vector.tensor_copy(out=tmpA, in_=otA[0:64, :])
            # head B's O hops to partitions 64:128 here, off the critical
            # path (runs in parallel with the reciprocal), so norm_back's
            # multiply can write onp[64:128] directly
            tmpB2 = wpool.tile([P, NI], bf16, tag="tmpB2", name="tmpB2", bufs=2)
            hop.dma_start(tmpB2[64:128, :], tmpB)
            return (pt, ic, zi0, tmpA, tmpB2, last)

        def emit_norm_back(st):
            """1/Z broadcast via two tiny PE matmuls (ones-column x zi0 row
            -> PSUM rows 0:64 / 64:128; ~215ns each) + the normalization
            multiplies, both writing onp directly. Emitted AFTER the chunk's
            matmuls so the in-order PE reaches the broadcasts only once zi0
            is long since ready."""
            pt, ic, zi0, tmpA, tmpB2, last = st
            onp = wpool.tile([P, NI], bf16, tag=f"onp{pt}_{ic}", name="onp")
            zb_ps = gpool.tile([P, NI], f32, tag="work", name="zb_ps")
            nc.tensor.matmul(
                zb_ps[0:64, :],
                lhsT=ones_sb[0:1, 0:64],
                rhs=zi0[0:1, 0:NI],
                start=True,
                stop=True,
            )
            nc.tensor.matmul(
                zb_ps[64:128, :],
                lhsT=ones_sb[0:1, 0:64],
                rhs=zi0[0:1, NI : 2 * NI],
                start=True,
                stop=True,
            )
            nc.vector.tensor_mul(
                out=onp[64:128, :], in0=tmpB2[64:128, :], in1=zb_ps[64:128, :]
            )
            nc.vector.tensor_mul(out=onp[0:64, :], in0=tmpA, in1=zb_ps[0:64, :])
            onp_tiles[(pt, ic)] = onp

        def gen_outproj(ic):
            # osb casts on gpsimd (DVE carries the offloaded exps); the final
            # outproj spreads its out-DMAs over 4 queues (scalar/vector are
            # idle by then) so the last 1MB drains fast
            last = ic == NIC - 1
            for it_in in range(NI // P):
                for ec in range(2):
                    piece = 2 * it_in + ec
                    o_ps = gpool.tile([P, NI], f32, tag="work", name="o_ps")
                    for pt in range(2):
                        nc.tensor.matmul(
                            o_ps,
                            lhsT=onp_tiles[(pt, ic)][:, it_in * P : (it_in + 1) * P],
                            rhs=wo_sb[:, pt, ec * NI : (ec + 1) * NI],
                            start=(pt == 0),
                            stop=(pt == 1),
                        )
                        yield
                    osb = opool.tile([P, NI], bf16, tag="osb", name="osb")
                    if last:
                        nc.scalar.activation(
                            out=osb,
                            in_=o_ps,
                            func=mybir.ActivationFunctionType.Copy,
                        )
                    else:
                        nc.vector.tensor_copy(out=osb, in_=o_ps)
                    it = ic * (NI // P) + it_in
                    if last:
                        eng = (nc.sync, nc.gpsimd, nc.scalar)[(2 * it_in + ec) % 3]
                    else:
                        eng = (nc.sync, nc.gpsimd)[(it_in + ec) % 2]
                    eng.dma_start(
                        out[it * P : (it + 1) * P, ec * NI : (ec + 1) * NI], osb
                    )

        # ---- schedule ----
        # Upfront (PE-bound, scalar idle): V, pair-0 K, Q(0,0). Everything
        # else trickles in as interleaved fills inside the chunks, where exp
        # on the scalar engine (18.4us/chunk) outpaces the chunk's own
        # matmuls (13.7us): Q(0,ic+1)/K(1,*) during pair-0 chunks,
        # Q(1,ic+1)/outproj during pair-1 chunks. norm of chunk n is emitted
        # at the top of chunk n+1 (before any DVE copies of fills can block
        # it: DVE is strict FIFO); its outproj joins late_queue and is only
        # pumped in the second half of the next chunk, by which time the
        # norm chain has completed.
        # V(14), V(15) are consumed only from jt13 of chunk 0 — they ride
        # chunk 0's spare fill slots instead of the scalar-idle upfront phase
        for jt in range(NJT - 2):
            emit_proj_v(jt)
        emit_now(gen_proj_k(0, 0))
        emit_now(gen_proj_q(0, 0))
        for ic in range(1, NIC):
            emit_now(gen_proj_k(0, ic))

        chunks = [(0, ic) for ic in range(NIC)] + [(1, ic) for ic in range(NIC)]
        pending = None  # (pt, ic, otA, otB) awaiting norm front
        for n, (pt, ic) in enumerate(chunks):
            if n == 0:
                # deferred V tiles first: earliest deadline (E@V from jt13)
                fill_queue.append(gen_proj_v(NJT - 2))
                fill_queue.append(gen_proj_v(NJT - 1))
            if n < NIC:
                if n < NIC - 1:
                    fill_queue.append(gen_proj_q(0, n + 1))
                else:
                    fill_queue.append(gen_proj_q(1, 0))
                fill_queue.append(gen_proj_k(1, n))
            elif n < 2 * NIC - 1:
                fill_queue.append(gen_proj_q(1, n - NIC + 1))
            st = None
            if pending is not None:
                ppt, pic, pA, pB = pending
                st = emit_norm_front(ppt, pic, pA, pB)
            otA, otB = emit_chunk_mms(pt, ic)
            # norm_back AFTER the chunk: its PE broadcasts then sit behind
            # ~14us of chunk matmuls, by which time zi0 is ready
            if st is not None:
                emit_norm_back(st)
                if st[0] == 1:
                    late_queue.append(gen_outproj(st[1]))
            pending = (pt, ic, otA, otB)
        ppt, pic, pA, pB = pending
        st = emit_norm_front(ppt, pic, pA, pB, last=True)
        # leftover outproj pieces run on the PE while the chain front
        # (DVE/DMA) computes 1/Z
        drain_all()
        emit_norm_back(st)
        if st[0] == 1:
            late_queue.append(gen_outproj(st[1]))
        drain_all()


def _get_program():
    global _CACHED_NC
    if _CACHED_NC is None:
        _CACHED_NC = _build_program()
    return _CACHED_NC


def _shard_inputs(x, Wq, Wkv, Wo):
    bf = ml_dtypes.bfloat16
    xTs = [np.ascontiguousarray(x[b].T).astype(bf) for b in range(B)]
    wqs, wks, wvs, wos = [], [], [], []
    for hg in range(HG):
        c0 = hg * HG * DHEAD
        c1 = c0 + HG * DHEAD
        wqs.append(np.ascontiguousarray(Wq[:, c0:c1]).astype(bf))
        wks.append(np.ascontiguousarray(Wkv[:, c0:c1]).astype(bf))
        wvs.append(np.ascontiguousarray(Wkv[:, DIM + c0 : DIM + c1]).astype(bf))
        wos.append(np.ascontiguousarray(Wo[c0:c1, :]).astype(bf))
    in_maps = []
    for core in range(NCORES):
        b, hg = core // HG, core % HG
        in_maps.append(
            {
                "xT": xTs[b],
                "wq": wqs[hg],
                "wk": wks[hg],
                "wv": wvs[hg],
                "wo": wos[hg],
            }
        )
    return in_maps


def kernel(x, similarity, Wq, Wkv, Wo, bo):
    global LAST_RESULTS
    _ensure_profile_hook()
    import concourse.bass_utils as _bu
    from concourse.bass_utils import run_bass_kernel_spmd

    # keep trace artifacts local if profiling is ever enabled (no S3 here)
    _bu.upload_artifacts = lambda tmpdir: tmpdir

    x = np.asarray(x, dtype=np.float32)
    Wq = np.asarray(Wq, dtype=np.float32)
    Wkv = np.asarray(Wkv, dtype=np.float32)
    Wo = np.asarray(Wo, dtype=np.float32)
    bo = np.asarray(bo, dtype=np.float32)

    nc = _get_program()
    in_maps = _shard_inputs(x, Wq, Wkv, Wo)
    res = run_bass_kernel_spmd(nc, in_maps, list(range(NCORES)))
    LAST_RESULTS = res
    outs = [res.results[i]["out"] for i in range(NCORES)]
    full = np.empty((B, N, DIM), dtype=np.float32)
    for b in range(B):
        acc = outs[4 * b].astype(np.float32).copy()
        for hg in range(1, HG):
            acc += outs[4 * b + hg]
        full[b] = acc + bo[None, :]
    return full


def _sim_check():
    """Simulate core 0 on CoreSim and compare against numpy reference."""
    from concourse.bass_interp import CoreSim

    rng = np.random.default_rng(0)
    x = rng.standard_normal((B, N, DIM), dtype=np.float32)
    Wq = (rng.standard_normal((DIM, DIM), dtype=np.float32) * DIM**-0.5).astype(
        np.float32
    )
    Wkv = (
        rng.standard_normal((DIM, 2 * DIM), dtype=np.float32) * DIM**-0.5
    ).astype(np.float32)
    Wo = (rng.standard_normal((DIM, DIM), dtype=np.float32) * DIM**-0.5).astype(
        np.float32
    )

    nc = _get_program()
    in_maps = _shard_inputs(x, Wq, Wkv, Wo)
    core = 0
    sim = CoreSim(nc)
    for name, arr in in_maps[core].items():
        sim.tensor(name)[:] = arr
    sim.simulate()
    got = np.array(sim.tensor("out"))

    # numpy reference for core 0's partial (batch 0, heads 0-3), fp32 exact
    b, hg = 0, 0
    xb = x[b]
    q = xb @ Wq[:, hg * 256 : hg * 256 + 256]
    k = xb @ Wkv[:, hg * 256 : hg * 256 + 256]
    v = xb @ Wkv[:, DIM + hg * 256 : DIM + hg * 256 + 256]
    partial = np.zeros((N, DIM), dtype=np.float32)
    for h in range(HG):
        qh = q[:, h * 64 : h * 64 + 64]
        kh = k[:, h * 64 : h * 64 + 64]
        vh = v[:, h * 64 : h * 64 + 64]
        dots = (qh @ kh.T) * SCALE
        dots -= dots.max(axis=-1, keepdims=True)
        e = np.exp(dots)
        attn = e / e.sum(axis=-1, keepdims=True)
        partial += (attn @ vh) @ Wo[hg * 256 + h * 64 : hg * 256 + h * 64 + 64, :]

    err = np.abs(got - partial)
    scale = np.abs(partial).max()
    print("max abs err:", err.max(), "scale:", scale, "rel:", err.max() / scale)
    return err.max() / scale


if __name__ == "__main__":
    _sim_check()

